# revision 1
# baseline (speedup 1.0000x reference)
"""OIM loss with circular queue — Trainium2 Bass kernel (8 NeuronCores).

Strategy
--------
The output is a scalar:  loss = mean_b [ logsumexp_{q in good}(30*cos(x_b, e_q))
                                         - 30*cos(x_b, e_{xe_b}) ]
where e is the circular queue after the (sequential, data-dependent) update.

The queue update only *moves integer labels around* plus writes U=256
normalized per-pid mean embeddings into a contiguous window of slots.  All the
integer bookkeeping (which slots are invalidated, which slot each batch row
targets) is done on the host; every FLOP-heavy part runs on the 8 cores:

  - per-pid masked means  (one-hot mask matmul,  [U,B]x[B,D])
  - row normalization of inputs and means
  - the big logits matmul [B,Q//8,D] per core (float32r, full PE rate)
    fused with exp (ACT: exp(30*s - M), M a safe upper bound of the row max)
    and the masked row-sum (DVE tensor_tensor_reduce with the `good` mask)
  - target cosines via a small [B,U] matmul + one-hot gather

Sharding: emb_cq is sharded over Q (2048 rows/core, tensor-parallel); the
batch-side preprocessing is replicated (it is ~2% of the FLOPs).  Each core
returns partial sums of exp(logit - M) over its Q-shard plus the target
cosines; the host adds the 8 partials (the "allreduce"), takes log and means.
"""

import os
import sys

import numpy as np

for _p in ("/opt/trn_rl_repo", "/root/.axon_site/_ro/trn_rl_repo"):
    if os.path.isdir(_p) and _p not in sys.path:
        sys.path.insert(0, _p)

B, D, Q, U = 4096, 512, 16384, 256
N_CORES = 8
QS = Q // N_CORES          # queue rows per core
OIM_SCALAR = 30.0
IGNORE = -1
MT = B // 128              # 32 b-tiles
QT = QS // 128             # 16 q-tiles per core
KD = D // 128              # 4 contraction chunks
NQ = QS // 512             # 4 matmul n-chunks per core
UT = U // 128              # 2 u-tiles

_PROG_CACHE = {}


def _build_program(M: float, work_bufs=4, psm_bufs=2, kd_outer=False, pst_bufs=4, small_bufs=6, exp_bufs=6, tl_bufs=4):
    """Emit + schedule + compile the (SPMD, identical on all cores) program."""
    import concourse.bacc as bacc
    import concourse.bass as bass
    import concourse.tile as tile
    from concourse import mybir
    from concourse.masks import make_identity

    f32 = mybir.dt.float32
    f32r = mybir.dt.float32r
    AF = mybir.ActivationFunctionType
    OP = mybir.AluOpType

    nc = bacc.Bacc("TRN2", target_bir_lowering=False, debug=False,
                   num_devices=N_CORES)

    x_d = nc.dram_tensor("x", [B, D], f32, kind="ExternalInput").ap()
    emb_d = nc.dram_tensor("emb", [QS, D], f32, kind="ExternalInput").ap()
    labf_d = nc.dram_tensor("labf", [128, MT], f32, kind="ExternalInput").ap()
    uniqf_d = nc.dram_tensor("uniqf", [128, U], f32, kind="ExternalInput").ap()
    cnts_d = nc.dram_tensor("cnts", [128, UT], f32, kind="ExternalInput").ap()
    widx_d = nc.dram_tensor("widx", [128, MT], f32, kind="ExternalInput").ap()
    iota_d = nc.dram_tensor("iota", [128, U], f32, kind="ExternalInput").ap()
    gkeep_d = nc.dram_tensor("gkeep", [128, QS], f32, kind="ExternalInput").ap()
    wkeep_d = nc.dram_tensor("wkeep", [128, QT], f32, kind="ExternalInput").ap()
    oht_d = nc.dram_tensor("oht", [128, UT, QS], f32, kind="ExternalInput").ap()
    sume_d = nc.dram_tensor("sume", [128, MT], f32, kind="ExternalOutput").ap()
    tco_d = nc.dram_tensor("tco", [128, MT], f32, kind="ExternalOutput").ap()
    tick_d = nc.dram_tensor("tick", [128, 4], f32, kind="ExternalInput").ap()
    tock_d = nc.dram_tensor("tock", [128, 4], f32, kind="ExternalOutput").ap()

    with tile.TileContext(nc) as tc:
        with (
            tc.tile_pool(name="singles", bufs=1) as singles,
            tc.tile_pool(name="work", bufs=work_bufs) as work,
            tc.tile_pool(name="small", bufs=small_bufs) as small,
            tc.tile_pool(name="psum_t", bufs=pst_bufs, space="PSUM") as psum_t,
        ):
            # ---------------- constants / small inputs ----------------
            ident = singles.tile([128, 128], f32)
            make_identity(nc, ident)

            labs = singles.tile([128, MT], f32)
            nc.sync.dma_start(out=labs, in_=labf_d)
            widx = singles.tile([128, MT], f32)
            nc.sync.dma_start(out=widx, in_=widx_d)
            wkp = singles.tile([128, QT], f32)
            nc.sync.dma_start(out=wkp, in_=wkeep_d)
            cnts = singles.tile([128, UT], f32)
            nc.sync.dma_start(out=cnts, in_=cnts_d)
            uniqb = singles.tile([128, U], f32)
            nc.sync.dma_start(out=uniqb, in_=uniqf_d)
            iotab = singles.tile([128, U], f32)
            nc.sync.dma_start(out=iotab, in_=iota_d)
            keepg = singles.tile([128, QS], f32)
            nc.sync.dma_start(out=keepg, in_=gkeep_d)
            oht = singles.tile([128, UT, QS], f32r)
            nc.sync.dma_start(out=oht, in_=oht_d.bitcast(f32r))

            rcnt = singles.tile([128, UT], f32)
            nc.vector.reciprocal(rcnt, cnts)
            biasM = singles.tile([128, 1], f32)
            nc.vector.memset(biasM, -M)

            # resident big tensors
            xn_all = singles.tile([128, MT, D], f32)     # normalized inputs (b-major)
            embT = singles.tile([128, KD, QS], f32r)     # blended emb, d-major
            uembT = singles.tile([128, KD, U], f32r)     # uniq means, d-major
            uemb_n = singles.tile([128, UT, D], f32r)    # uniq means, u-major
            ssb = singles.tile([128, MT], f32)           # sum-exp out collector
            tsb = singles.tile([128, MT], f32)           # target-cos out collector

            # ---------------- phase 1+2: masked means + normalize ----------
            with tc.tile_pool(name="psum_u", bufs=1, space="PSUM") as psum_u:
                ps_u = [psum_u.tile([128, D], f32, tag=f"uniq{mu}",
                                    name=f"ps_u{mu}") for mu in range(UT)]
                for i in range(MT):
                    x_raw = work.tile([128, D], f32r, tag="x_raw")
                    nc.sync.dma_start(out=x_raw,
                                      in_=x_d[i * 128:(i + 1) * 128, :].bitcast(f32r))
                    x_f = x_raw.bitcast(f32)

                    # mask[b, u] = (uniq[u] == labels[b])
                    mt_ = work.tile([128, U], f32r, tag="maskr")
                    nc.vector.tensor_scalar(out=mt_, in0=uniqb,
                                            scalar1=labs[:, i:i + 1], scalar2=None,
                                            op0=OP.is_equal)
                    for mu in range(UT):
                        nc.tensor.matmul(ps_u[mu],
                                         mt_[:, mu * 128:(mu + 1) * 128],
                                         x_raw, start=(i == 0),
                                         stop=(i == MT - 1))

                    # row-normalize x
                    sq = work.tile([128, D], f32, tag="sq")
                    ssq = small.tile([128, 1], f32, tag="ssq")
                    nc.vector.scalar_tensor_tensor(out=sq, in0=x_f, scalar=1.0,
                                                   in1=x_f, op0=OP.mult,
                                                   op1=OP.mult, accum_out=ssq)
                    nrm = small.tile([128, 1], f32, tag="nrm")
                    nc.scalar.activation(out=nrm, in_=ssq, func=AF.Sqrt)
                    nc.vector.tensor_scalar_max(out=nrm, in0=nrm, scalar1=1e-12)
                    rin = small.tile([128, 1], f32, tag="rin")
                    nc.vector.reciprocal(rin, nrm)
                    nc.vector.tensor_scalar_mul(out=xn_all[:, i, :], in0=x_f,
                                                scalar1=rin)

                # finish uniq means: mean, normalize, transpose to d-major
                for mu in range(UT):
                    ue = uemb_n[:, mu, :]
                    nc.vector.tensor_scalar_mul(out=ue, in0=ps_u[mu],
                                                scalar1=rcnt[:, mu:mu + 1])
                    sq2 = work.tile([128, D], f32, tag="sq")
                    ssq2 = small.tile([128, 1], f32, tag="ssq")
                    ue_f = ue.bitcast(f32)
                    nc.vector.scalar_tensor_tensor(out=sq2, in0=ue_f, scalar=1.0,
                                                   in1=ue_f, op0=OP.mult,
                                                   op1=OP.mult, accum_out=ssq2)
                    nrm2 = small.tile([128, 1], f32, tag="nrm")
                    nc.scalar.activation(out=nrm2, in_=ssq2, func=AF.Sqrt)
                    nc.vector.tensor_scalar_max(out=nrm2, in0=nrm2, scalar1=1e-12)
                    rin2 = small.tile([128, 1], f32, tag="rin")
                    nc.vector.reciprocal(rin2, nrm2)
                    nc.vector.tensor_scalar_mul(out=ue, in0=ue_f, scalar1=rin2)
                    for kd in range(KD):
                        pst = psum_t.tile([128, 128], f32, tag="pst")
                        nc.tensor.transpose(
                            pst,
                            uemb_n[:, mu, kd * 128:(kd + 1) * 128].bitcast(f32),
                            ident)
                        nc.scalar.copy(out=uembT[:, kd, mu * 128:(mu + 1) * 128],
                                       in_=pst)

            # ---------------- phase 3: blend queue window + transpose ------
            with tc.tile_pool(name="psum_b", bufs=2, space="PSUM") as psum_b:
                for t in range(QT):
                    e_raw = work.tile([128, D], f32, tag="e_raw")
                    nc.sync.dma_start(out=e_raw,
                                      in_=emb_d[t * 128:(t + 1) * 128, :])
                    eff = work.tile([128, D], f32, tag="eff")
                    # zero the window rows ...
                    nc.vector.tensor_scalar_mul(out=eff, in0=e_raw,
                                                scalar1=wkp[:, t:t + 1])
                    # ... and add one-hot @ uniq_means
                    psb = psum_b.tile([128, D], f32, tag="psb")
                    for ku in range(UT):
                        nc.tensor.matmul(psb,
                                         oht[:, ku, t * 128:(t + 1) * 128],
                                         uemb_n[:, ku, :],
                                         start=(ku == 0), stop=(ku == UT - 1))
                    nc.vector.tensor_add(out=eff, in0=eff, in1=psb)
                    for kd in range(KD):
                        pst = psum_t.tile([128, 128], f32, tag="pst")
                        nc.tensor.transpose(pst, eff[:, kd * 128:(kd + 1) * 128],
                                            ident)
                        nc.scalar.copy(out=embT[:, kd, t * 128:(t + 1) * 128],
                                       in_=pst)

            # ---------------- phase 4: logits + fused LSE ----------------
            with (
                tc.tile_pool(name="psum_s", bufs=2, space="PSUM") as psum_s,
                tc.tile_pool(name="psum_m", bufs=psm_bufs, space="PSUM") as psum_m,
            ):
                for m in range(MT):
                    tl = work.tile([128, D], f32r, tag="lhsT", bufs=tl_bufs)
                    for kd in range(KD):
                        pst = psum_t.tile([128, 128], f32, tag="pst")
                        nc.tensor.transpose(
                            pst, xn_all[:, m, kd * 128:(kd + 1) * 128], ident)
                        nc.scalar.copy(out=tl[:, kd * 128:(kd + 1) * 128], in_=pst)

                    # target cosines: S2[b, u] then one-hot gather along u
                    pss = psum_s.tile([128, U], f32, tag="pss")
                    for kd in range(KD):
                        nc.tensor.matmul(pss, tl[:, kd * 128:(kd + 1) * 128],
                                         uembT[:, kd, :],
                                         start=(kd == 0), stop=(kd == KD - 1))
                    scr_u = work.tile([128, U], f32, tag="mask")
                    nc.vector.scalar_tensor_tensor(out=scr_u, in0=iotab,
                                                   scalar=widx[:, m:m + 1],
                                                   in1=pss,
                                                   op0=OP.is_equal, op1=OP.mult,
                                                   accum_out=tsb[:, m:m + 1])

                    # big matmul over this core's Q-shard, fused exp+masked sum
                    acc4 = small.tile([128, NQ], f32, tag="acc4")
                    if kd_outer:
                        psms = [psum_m.tile([128, 512], f32, tag=f"psm{n}",
                                            name=f"psm_{m}_{n}") for n in range(NQ)]
                        for kd in range(KD):
                            for n in range(NQ):
                                nc.tensor.matmul(
                                    psms[n], tl[:, kd * 128:(kd + 1) * 128],
                                    embT[:, kd, n * 512:(n + 1) * 512],
                                    start=(kd == 0), stop=(kd == KD - 1))
                        for n in range(NQ):
                            expt = work.tile([128, 512], f32, tag="expt", bufs=exp_bufs)
                            nc.scalar.activation(out=expt, in_=psms[n], func=AF.Exp,
                                                 bias=biasM, scale=OIM_SCALAR)
                            scr = work.tile([128, 512], f32, tag="scr", bufs=exp_bufs)
                            nc.vector.scalar_tensor_tensor(
                                out=scr, in0=expt, scalar=1.0,
                                in1=keepg[:, n * 512:(n + 1) * 512],
                                op0=OP.mult, op1=OP.mult,
                                accum_out=acc4[:, n:n + 1])
                    else:
                        for n in range(NQ):
                            psm = psum_m.tile([128, 512], f32, tag="psm")
                            for kd in range(KD):
                                nc.tensor.matmul(
                                    psm, tl[:, kd * 128:(kd + 1) * 128],
                                    embT[:, kd, n * 512:(n + 1) * 512],
                                    start=(kd == 0), stop=(kd == KD - 1))
                            expt = work.tile([128, 512], f32, tag="expt", bufs=exp_bufs)
                            nc.scalar.activation(out=expt, in_=psm, func=AF.Exp,
                                                 bias=biasM, scale=OIM_SCALAR)
                            scr = work.tile([128, 512], f32, tag="scr", bufs=exp_bufs)
                            nc.vector.scalar_tensor_tensor(
                                out=scr, in0=expt, scalar=1.0,
                                in1=keepg[:, n * 512:(n + 1) * 512],
                                op0=OP.mult, op1=OP.mult,
                                accum_out=acc4[:, n:n + 1])
                    nc.vector.reduce_sum(out=ssb[:, m:m + 1], in_=acc4,
                                         axis=mybir.AxisListType.X)

            nc.sync.dma_start(out=sume_d, in_=ssb)
            nc.sync.dma_start(out=tco_d, in_=tsb)
            tickt = singles.tile([128, 4], f32)
            nc.sync.dma_start(out=tickt, in_=tick_d)
            nc.sync.dma_start(out=tock_d, in_=tickt)

    nc.compile()
    return nc


def _host_bookkeeping(labels, label_cq, header_cq):
    """Mirror the reference's integer-only queue-update semantics."""
    labels = np.asarray(labels).astype(np.int64)
    lab = np.asarray(label_cq).astype(np.int64).copy()
    h0 = int(np.asarray(header_cq))

    # jnp.unique(labels, size=U): sorted unique, padded with the minimum
    uq = np.unique(labels)
    if uq.size < U:
        uniq = np.concatenate([uq, np.full(U - uq.size, uq.min(), np.int64)])
    else:
        uniq = uq[:U]
    cnts = np.array([(labels == v).sum() for v in uniq], np.int64)

    emb_src = np.full(Q, -1, np.int64)   # >=0: row u of uniq means; -1: original
    h = h0 % Q
    for u in range(U):
        y = uniq[u]
        m = lab == y
        i = int(np.argmax(m)) if m.any() else 0
        inval = bool(m.any()) and (i != h)
        emb_src[h] = u
        lab[h] = y
        if inval:
            lab[i] = IGNORE
        h = (h + 1) % Q

    good = lab != IGNORE
    goodidx = np.flatnonzero(good)
    gl = lab[goodidx]
    vals, first = np.unique(gl, return_index=True)
    pos = np.searchsorted(vals, labels)
    assert np.all(vals[np.clip(pos, 0, vals.size - 1)] == labels), \
        "batch label missing from queue"
    xe = goodidx[first[pos]]
    return uniq, cnts, emb_src, good, xe


def _prepare(inputs, labels, emb_cq, label_cq, header_cq):
    """Host bookkeeping -> (M, per-core input maps, extra-target indices, xe)."""
    inputs = np.ascontiguousarray(np.asarray(inputs, np.float32))
    emb_cq = np.ascontiguousarray(np.asarray(emb_cq, np.float32))

    uniq, cnts, emb_src, good, xe = _host_bookkeeping(labels, label_cq, header_cq)

    # safe upper bound for any logit: 30 * max row norm (uniq means have norm 1)
    max_nrm = float(np.sqrt((emb_cq.astype(np.float64) ** 2).sum(axis=1).max()))
    M = OIM_SCALAR * max(1.0, max_nrm) * 1.0000001

    w_idx = emb_src[xe].astype(np.float64)        # -1 for non-window targets
    extra = np.flatnonzero(w_idx < 0)             # handled on host (rare/none)

    def pmajor(v, cols):
        return np.ascontiguousarray(
            np.asarray(v, np.float32).reshape(cols, 128).T)

    base = {
        "x": inputs,
        "tick": np.zeros((128, 4), np.float32),
        "labf": pmajor(np.asarray(labels, np.float64), MT),
        "uniqf": np.ascontiguousarray(
            np.broadcast_to(uniq.astype(np.float32), (128, U))),
        "cnts": pmajor(cnts, UT),
        "widx": pmajor(w_idx, MT),
        "iota": np.ascontiguousarray(
            np.broadcast_to(np.arange(U, dtype=np.float32), (128, U))),
    }
    in_maps = []
    for c in range(N_CORES):
        sl = slice(c * QS, (c + 1) * QS)
        src = emb_src[sl]
        ohtT = np.zeros((U, QS), np.float32)
        j = np.flatnonzero(src >= 0)
        ohtT[src[j], j] = 1.0
        in_maps.append({
            **base,
            "emb": np.ascontiguousarray(emb_cq[sl]),
            "gkeep": np.ascontiguousarray(
                np.broadcast_to(good[sl].astype(np.float32), (128, QS))),
            "wkeep": pmajor((src < 0).astype(np.float32), QT),
            "oht": np.ascontiguousarray(
                ohtT.reshape(UT, 128, QS).transpose(1, 0, 2)),
        })
    return M, in_maps, extra, xe


def _combine(res_list, M, extra, xe, inputs, emb_cq):
    """Unshard / combine per-core partials into the scalar loss."""
    S = np.zeros(B, np.float64)
    for r in res_list:
        S += r["sume"].astype(np.float64).T.reshape(B)
    t_cos = res_list[0]["tco"].astype(np.float64).T.reshape(B)

    if extra.size:  # targets pointing at original (non-window) queue rows
        xb = np.asarray(inputs, np.float64)[extra]
        xb /= np.maximum(np.linalg.norm(xb, axis=1, keepdims=True), 1e-12)
        eb = np.asarray(emb_cq, np.float64)[xe[extra]]
        t_cos[extra] = (xb * eb).sum(axis=1)

    loss = np.mean(M + np.log(S) - OIM_SCALAR * t_cos)
    return np.array(loss, dtype=np.float32)


def kernel(inputs, labels, emb_cq, label_cq, age_cq, header_cq):
    from concourse.bass_utils import run_bass_kernel_spmd

    M, in_maps, extra, xe = _prepare(inputs, labels, emb_cq, label_cq, header_cq)

    key = round(M, 9)
    if key not in _PROG_CACHE:
        _PROG_CACHE[key] = _build_program(M)
    nc = _PROG_CACHE[key]

    res = run_bass_kernel_spmd(nc, in_maps, core_ids=list(range(N_CORES)))
    return _combine(res.results, M, extra, xe, inputs, emb_cq)



# revision 5
# speedup vs baseline: 1.9160x; 1.9160x over previous
"""OIM loss with circular queue — Trainium2 Bass kernel (8 NeuronCores).

Strategy (v2, fp8 DoubleRow)
----------------------------
loss = mean_b [ M + log S_b - 30*cos(x_b, e_{xe_b}) ],
S_b = sum_{q good} exp(30*cos(x_b, e_q) - M), with e the post-update queue.

Device-side compute per core (tensor-parallel over Q):
  - per-pid masked means (one-hot mask matmul in bf16, fp32 PSUM accum)
  - row norms of x (bf16 squares, fp32 accum)
  - normalized means -> fp8 d-major tiles (PE transpose + gated cast)
  - big logits matmul in fp8e4 with DoubleRow perf mode (K=256/pass),
    fused exp via ACT activation (per-row scale=30*rin/SX, bias=-M) with
    the row-sum taken by the activation accumulator
  - target cosines from a small fp8-DR matmul against the mean tiles

Layout trick: the 256 queue slots rewritten by the circular-queue update
("window") are assigned to core 0's shard as its first 256 columns; the
other 16128 original slots fill the rest.  Bad slots (label IGNORE) are
zero columns -> each contributes exactly exp(-M), subtracted on the host.
All integer bookkeeping, input layout (x^T / emb^T fp8 quantization) and
the final log/mean run on the host; all O(B*D*Q) FLOPs run on device.
"""

import os
import sys

import numpy as np

for _p in ("/opt/trn_rl_repo", "/root/.axon_site/_ro/trn_rl_repo"):
    if os.path.isdir(_p) and _p not in sys.path:
        sys.path.insert(0, _p)

import ml_dtypes

B, D, Q, U = 4096, 512, 16384, 256
N_CORES = 8
QS = Q // N_CORES          # queue columns per core
W0 = U                     # window block size on core 0's layout
OIM_SCALAR = 30.0
IGNORE = -1
SXE = 16.0                 # fp8 scale for emb/mean operands
MT = B // 128              # 32 b-tiles
KP = 2                     # k-passes of 256 (DoubleRow)
KI = 2                     # interleave factor inside a pass
UT = U // 128              # 2 u-tiles
NQ = QS // 512             # 4 matmul n-chunks per core
XCH = 4                    # x b-tiles per DMA chunk

F8 = ml_dtypes.float8_e4m3
BF = ml_dtypes.bfloat16

_PROG_CACHE = {}


def _build_program(M: float):
    import concourse.bacc as bacc
    import concourse.tile as tile
    from concourse import mybir
    from concourse.masks import make_identity

    f32 = mybir.dt.float32
    bf16 = mybir.dt.bfloat16
    fp8 = mybir.dt.float8e4
    AF = mybir.ActivationFunctionType
    OP = mybir.AluOpType
    DR = mybir.MatmulPerfMode.DoubleRow

    nc = bacc.Bacc("TRN2", target_bir_lowering=False, debug=False,
                   num_devices=N_CORES)

    xbf_d = nc.dram_tensor("xbf", [128, MT * D], bf16, kind="ExternalInput").ap()
    xT8_d = nc.dram_tensor("xT8", [128, KP * KI * B], fp8, kind="ExternalInput").ap()
    eT8_d = nc.dram_tensor("eT8", [128, KP * KI * QS], fp8, kind="ExternalInput").ap()
    labf_d = nc.dram_tensor("labf", [128, MT], f32, kind="ExternalInput").ap()
    uniq_d = nc.dram_tensor("uniqb", [128, U], bf16, kind="ExternalInput").ap()
    rcnt_d = nc.dram_tensor("rcntf", [128, UT], f32, kind="ExternalInput").ap()
    gsx_d = nc.dram_tensor("gsxe", [128, 1], f32, kind="ExternalInput").ap()
    ukp_d = nc.dram_tensor("ukeepf", [128, UT], f32, kind="ExternalInput").ap()
    widx_d = nc.dram_tensor("widxf", [128, MT], f32, kind="ExternalInput").ap()
    iota_d = nc.dram_tensor("iotau", [128, U], f32, kind="ExternalInput").ap()
    osum_d = nc.dram_tensor("osum", [128, MT], f32, kind="ExternalOutput").ap()
    tco_d = nc.dram_tensor("tco", [128, MT], f32, kind="ExternalOutput").ap()

    with tile.TileContext(nc) as tc:
        with (
            tc.tile_pool(name="singles", bufs=1) as singles,
            tc.tile_pool(name="work", bufs=3) as work,
            tc.tile_pool(name="small", bufs=4) as small,
        ):
            ident = singles.tile([128, 128], bf16)
            make_identity(nc, ident)

            labs = singles.tile([128, MT], f32)
            nc.sync.dma_start(out=labs, in_=labf_d)
            uniqb = singles.tile([128, U], bf16)
            nc.sync.dma_start(out=uniqb, in_=uniq_d)
            rcnt = singles.tile([128, UT], f32)
            nc.sync.dma_start(out=rcnt, in_=rcnt_d)
            gsx = singles.tile([128, 1], f32)
            nc.sync.dma_start(out=gsx, in_=gsx_d)
            ukp = singles.tile([128, UT], f32)
            nc.sync.dma_start(out=ukp, in_=ukp_d)
            widx = singles.tile([128, MT], f32)
            nc.sync.dma_start(out=widx, in_=widx_d)
            iotab = singles.tile([128, U], f32)
            nc.sync.dma_start(out=iotab, in_=iota_d)

            # big resident operands (streamed in while phase A runs)
            xT8 = singles.tile([128, KP, KI, B], fp8)
            nc.sync.dma_start(out=xT8, in_=xT8_d)
            eT8 = singles.tile([128, KP, KI, QS], fp8)
            nc.sync.dma_start(out=eT8, in_=eT8_d)

            biasM = singles.tile([128, 1], f32)
            nc.vector.memset(biasM, -M)

            uembT8 = singles.tile([128, KP, KI, U], fp8)   # gated fp8 means^T
            nsq = singles.tile([128, MT], f32)             # |x_b|^2
            rin30 = singles.tile([128, MT], f32)           # 30/(SXE*|x_b|)
            macc = singles.tile([128, UT], f32)            # |mean_u|^2
            osum = singles.tile([128, MT], f32)            # sum-exp collector
            tco = singles.tile([128, MT], f32)             # target-cos collector

            # ---------- phase A: masked means + row norms ----------
            with tc.tile_pool(name="psum_u", bufs=1, space="PSUM") as psum_u:
                ps_u = [psum_u.tile([128, D], f32, tag=f"uniq{mu}",
                                    name=f"ps_u{mu}") for mu in range(UT)]
                for ch in range(MT // XCH):
                    xch = work.tile([128, XCH, D], bf16, tag="xch")
                    nc.sync.dma_start(
                        out=xch,
                        in_=xbf_d[:, ch * XCH * D:(ch + 1) * XCH * D])
                    for j in range(XCH):
                        m = ch * XCH + j
                        xm = xch[:, j, :]
                        mk = work.tile([128, U], bf16, tag="mask")
                        nc.vector.tensor_scalar(out=mk, in0=uniqb,
                                                scalar1=labs[:, m:m + 1],
                                                scalar2=None, op0=OP.is_equal)
                        for mu in range(UT):
                            nc.tensor.matmul(ps_u[mu],
                                             mk[:, mu * 128:(mu + 1) * 128],
                                             xm, start=(m == 0),
                                             stop=(m == MT - 1))
                        sq = work.tile([128, D], bf16, tag="sq")
                        nc.vector.scalar_tensor_tensor(
                            out=sq, in0=xm, scalar=1.0, in1=xm,
                            op0=OP.mult, op1=OP.mult,
                            accum_out=nsq[:, m:m + 1])

                # norm finalize (batched)
                nrm = small.tile([128, MT], f32, tag="nrm")
                nc.scalar.activation(out=nrm, in_=nsq, func=AF.Sqrt)
                nc.vector.tensor_scalar_max(out=nrm, in0=nrm, scalar1=1e-12)
                rinv = small.tile([128, MT], f32, tag="rinv")
                nc.vector.reciprocal(rinv, nrm)
                nc.vector.tensor_scalar_mul(out=rin30, in0=rinv,
                                            scalar1=OIM_SCALAR / SXE)

                # means: psum -> bf16, squared norms
                mean_bf = [singles.tile([128, D], bf16, name=f"mean_bf{mu}")
                           for mu in range(UT)]
                for mu in range(UT):
                    nc.vector.tensor_scalar_mul(out=mean_bf[mu], in0=ps_u[mu],
                                                scalar1=rcnt[:, mu:mu + 1])
                    sq2 = work.tile([128, D], bf16, tag="sq")
                    nc.vector.scalar_tensor_tensor(
                        out=sq2, in0=mean_bf[mu], scalar=1.0, in1=mean_bf[mu],
                        op0=OP.mult, op1=OP.mult,
                        accum_out=macc[:, mu:mu + 1])

            # ---------- mean normalize + transpose to fp8 d-major ----------
            with tc.tile_pool(name="psum_t", bufs=2, space="PSUM") as psum_t:
                mnr = small.tile([128, UT], f32, tag="mnr")
                nc.scalar.activation(out=mnr, in_=macc, func=AF.Sqrt)
                nc.vector.tensor_scalar_max(out=mnr, in0=mnr, scalar1=1e-12)
                mrc = small.tile([128, UT], f32, tag="mrc")
                nc.vector.reciprocal(mrc, mnr)
                # rmg = SXE*gate*ukeep/|mean| (per-partition u scale)
                rmg = small.tile([128, UT], f32, tag="rmg")
                nc.vector.tensor_scalar(out=rmg, in0=mrc,
                                        scalar1=gsx[:, 0:1], scalar2=None,
                                        op0=OP.mult)
                nc.vector.tensor_tensor(out=rmg, in0=rmg, in1=ukp,
                                        op=OP.mult)
                for mu in range(UT):
                    mng = work.tile([128, D], bf16, tag="mng")
                    nc.vector.tensor_scalar_mul(out=mng, in0=mean_bf[mu],
                                                scalar1=rmg[:, mu:mu + 1])
                    pst = psum_t.tile([128, D], bf16, tag="pst")
                    for kd in range(D // 128):
                        nc.tensor.transpose(pst[:, kd * 128:(kd + 1) * 128],
                                            mng[:, kd * 128:(kd + 1) * 128],
                                            ident)
                    nc.vector.tensor_scalar_mul(
                        out=uembT8[:, :, :, mu * 128:(mu + 1) * 128],
                        in0=pst, scalar1=1.0)
                # merge gated means into the first W0 columns of eT8
                ev = eT8[:, :, :, 0:U]
                nc.vector.scalar_tensor_tensor(
                    out=ev, in0=uembT8, scalar=1.0, in1=ev,
                    op0=OP.mult, op1=OP.add)

            # ---------- phase S2: target cosines ----------
            with tc.tile_pool(name="psum_s", bufs=2, space="PSUM") as psum_s:
                for m in range(MT):
                    ps2 = psum_s.tile([128, 512], f32, tag="ps2")
                    v = ps2[:, 0:U]
                    for kp in range(KP):
                        nc.tensor.matmul(v,
                                         xT8[:, kp, :, m * 128:(m + 1) * 128],
                                         uembT8[:, kp, :, :],
                                         start=(kp == 0), stop=(kp == KP - 1),
                                         perf_mode=DR)
                    scr = work.tile([128, U], f32, tag="scr")
                    nc.vector.scalar_tensor_tensor(
                        out=scr, in0=iotab, scalar=widx[:, m:m + 1], in1=v,
                        op0=OP.is_equal, op1=OP.mult,
                        accum_out=tco[:, m:m + 1])
                nc.vector.tensor_tensor(out=tco, in0=tco, in1=rin30,
                                        op=OP.mult)

            # ---------- phase C: logits + fused exp/sum ----------
            with tc.tile_pool(name="psum_l", bufs=2, space="PSUM") as psum_l:
                for m in range(MT):
                    pl = psum_l.tile([128, NQ * 512], f32, tag="pl")
                    for n in range(NQ):
                        for kp in range(KP):
                            nc.tensor.matmul(
                                pl[:, n * 512:(n + 1) * 512],
                                xT8[:, kp, :, m * 128:(m + 1) * 128],
                                eT8[:, kp, :, n * 512:(n + 1) * 512],
                                start=(kp == 0), stop=(kp == KP - 1),
                                perf_mode=DR)
                    ex = work.tile([128, NQ * 512], bf16, tag="ex", bufs=2)
                    nc.scalar.activation(out=ex, in_=pl, func=AF.Exp,
                                         bias=biasM, scale=rin30[:, m:m + 1],
                                         accum_out=osum[:, m:m + 1])

            nc.sync.dma_start(out=osum_d, in_=osum)
            nc.sync.dma_start(out=tco_d, in_=tco)

    nc.compile()
    return nc


def _host_bookkeeping(labels, label_cq, header_cq):
    """Mirror the reference's integer-only queue-update semantics."""
    labels = np.asarray(labels).astype(np.int64)
    lab = np.asarray(label_cq).astype(np.int64).copy()
    h0 = int(np.asarray(header_cq))

    uq = np.unique(labels)
    if uq.size < U:
        uniq = np.concatenate([uq, np.full(U - uq.size, uq.min(), np.int64)])
    else:
        uniq = uq[:U]
    cnts = np.array([(labels == v).sum() for v in uniq], np.int64)

    emb_src = np.full(Q, -1, np.int64)   # >=0: row u of uniq means; -1: original
    h = h0 % Q
    for u in range(U):
        y = uniq[u]
        m = lab == y
        i = int(np.argmax(m)) if m.any() else 0
        inval = bool(m.any()) and (i != h)
        emb_src[h] = u
        lab[h] = y
        if inval:
            lab[i] = IGNORE
        h = (h + 1) % Q

    good = lab != IGNORE
    goodidx = np.flatnonzero(good)
    gl = lab[goodidx]
    vals, first = np.unique(gl, return_index=True)
    pos = np.searchsorted(vals, labels)
    assert np.all(vals[np.clip(pos, 0, vals.size - 1)] == labels), \
        "batch label missing from queue"
    xe = goodidx[first[pos]]
    return uniq, cnts, emb_src, good, xe


def _pmajor(v, cols):
    return np.ascontiguousarray(np.asarray(v, np.float32).reshape(cols, 128).T)


def _prepare(inputs, labels, emb_cq, label_cq, header_cq):
    inputs = np.asarray(inputs, np.float32)
    emb_cq = np.asarray(emb_cq, np.float32)

    uniq, cnts, emb_src, good, xe = _host_bookkeeping(labels, label_cq, header_cq)

    max_nrm = float(np.sqrt((emb_cq.astype(np.float64) ** 2).sum(axis=1).max()))
    M = OIM_SCALAR * max(1.0, max_nrm) * 1.0000001

    window = emb_src >= 0
    u_slot = np.full(U, -1, np.int64)
    wi = np.flatnonzero(window)
    u_slot[emb_src[wi]] = wi
    u_kept = (u_slot >= 0) & good[np.clip(u_slot, 0, Q - 1)]

    w_idx = emb_src[xe].astype(np.float64)        # -1 for non-window targets
    w_idx[w_idx >= 0] = np.where(
        u_kept[w_idx[w_idx >= 0].astype(np.int64)],
        w_idx[w_idx >= 0], -1.0)
    extra = np.flatnonzero(w_idx < 0)             # handled on host (rare/none)

    # ---- device input layouts ----
    xbf = np.ascontiguousarray(
        inputs.reshape(MT, 128, D).transpose(1, 0, 2).reshape(128, MT * D)
    ).astype(BF)
    # xT8[p, kp, i, b] = fp8(x[b, 256*kp + 128*i + p])
    xT = inputs.T.astype(F8)                      # [D, B]
    xT8 = np.ascontiguousarray(
        xT.reshape(KP, KI, 128, B).transpose(2, 0, 1, 3).reshape(128, -1))

    keep_orig = good & ~window
    embq = (SXE * emb_cq).astype(F8)
    embq[~keep_orig] = 0                          # bad or window -> zero cols
    orig_idx = np.flatnonzero(~window)            # Q-U slots, canonical order
    n_orig0 = QS - W0                             # originals on core 0

    nzero = int((~keep_orig[orig_idx]).sum()) + int((~u_kept).sum())

    base = {
        "xbf": xbf,
        "xT8": xT8,
        "labf": _pmajor(np.asarray(labels, np.float64), MT),
        "uniqb": np.ascontiguousarray(
            np.broadcast_to(uniq.astype(BF), (128, U))),
        "rcntf": _pmajor(1.0 / cnts.astype(np.float64), UT),
        "ukeepf": _pmajor(u_kept.astype(np.float64), UT),
        "widxf": _pmajor(w_idx, MT),
        "iotau": np.ascontiguousarray(
            np.broadcast_to(np.arange(U, dtype=np.float32), (128, U))),
    }

    def to_dmajor(cols):
        # cols: [QS, D] fp8 -> [128, KP*KI*QS] with (p,kp,i,j) layout
        t = np.ascontiguousarray(cols).T          # [D, QS]
        return np.ascontiguousarray(
            t.reshape(KP, KI, 128, QS).transpose(2, 0, 1, 3).reshape(128, -1))

    in_maps = []
    for c in range(N_CORES):
        cols = np.zeros((QS, D), F8)
        if c == 0:
            cols[W0:] = embq[orig_idx[:n_orig0]]
        else:
            sl = orig_idx[n_orig0 + (c - 1) * QS: n_orig0 + c * QS]
            cols[:] = embq[sl]
        gate = np.full((128, 1), SXE if c == 0 else 0.0, np.float32)
        in_maps.append({**base, "eT8": to_dmajor(cols), "gsxe": gate})
    return M, in_maps, extra, xe, nzero


def _combine(res_list, M, extra, xe, nzero, inputs, emb_cq):
    S = np.zeros(B, np.float64)
    for r in res_list:
        S += r["osum"].astype(np.float64).T.reshape(B)
    S -= nzero * np.exp(-np.float64(M))
    t30 = res_list[0]["tco"].astype(np.float64).T.reshape(B)

    if extra.size:  # targets pointing at original (non-window) queue rows
        xb = np.asarray(inputs, np.float64)[extra]
        xb /= np.maximum(np.linalg.norm(xb, axis=1, keepdims=True), 1e-12)
        eb = np.asarray(emb_cq, np.float64)[xe[extra]]
        t30[extra] = OIM_SCALAR * (xb * eb).sum(axis=1)

    loss = np.mean(M + np.log(S) - t30)
    return np.array(loss, dtype=np.float32)


def kernel(inputs, labels, emb_cq, label_cq, age_cq, header_cq):
    from concourse.bass_utils import run_bass_kernel_spmd

    M, in_maps, extra, xe, nzero = _prepare(
        inputs, labels, emb_cq, label_cq, header_cq)

    key = round(M, 9)
    if key not in _PROG_CACHE:
        _PROG_CACHE[key] = _build_program(M)
    nc = _PROG_CACHE[key]

    res = run_bass_kernel_spmd(nc, in_maps, core_ids=list(range(N_CORES)))
    return _combine(res.results, M, extra, xe, nzero, inputs, emb_cq)


# revision 7
# speedup vs baseline: 2.4227x; 1.2644x over previous
"""OIM loss with circular queue — Trainium2 Bass kernel (8 NeuronCores).

Strategy (v3, fp8 DoubleRow end-to-end)
---------------------------------------
loss = mean_b [ M + log S_b - 30*cos(x_b, e_{xe_b}) ],
S_b = sum_{q good} exp(30*cos(x_b, e_q) - M), with e the post-update queue.

Device-side compute per core (tensor-parallel over Q):
  - per-pid masked means: fp8 DoubleRow matmul (host-built one-hot masks)
  - row norms of x: fp8 DR matmul x@x^T per b-tile, diagonal extracted on DVE
  - normalized means -> gated fp8 d-major tiles (PE transpose + cast)
  - big logits matmul in fp8e4 DoubleRow (K=256/pass) into [128,2048] PSUM,
    exp on ACT (per-row scale=30*rin/SX, bias=-M), row-sum accumulated by a
    DVE tensor_scalar pass, target logits gathered from PSUM cols 0:256.

Layout trick: the 256 queue slots rewritten by the circular-queue update
("window") are core 0's first 256 columns; the other 16128 original slots
fill the rest (2048/core on cores 1-7).  Bad slots (label IGNORE) are zero
columns -> each contributes exactly exp(-M), subtracted on the host.  The
target slot of every batch row is a window slot, so its logit is read from
the first 256 PSUM columns on core 0 (cores 1-7 produce garbage there and
the host ignores it).  All integer bookkeeping, input layout (fp8
quantization / transposes) and the final log/mean run on the host; all
O(B*D*Q) FLOPs run on device.
"""

import os
import sys

import numpy as np

for _p in ("/opt/trn_rl_repo", "/root/.axon_site/_ro/trn_rl_repo"):
    if os.path.isdir(_p) and _p not in sys.path:
        sys.path.insert(0, _p)

import ml_dtypes

B, D, Q, U = 4096, 512, 16384, 256
N_CORES = 8
QS = Q // N_CORES          # queue columns per core
W0 = U                     # window block size on core 0's layout
OIM_SCALAR = 30.0
IGNORE = -1
SXE = 16.0                 # fp8 scale for emb/mean operands
MT = B // 128              # 32 b-tiles
KP = 2                     # k-passes of 256 (DoubleRow)
KI = 2                     # interleave factor inside a pass
UT = U // 128              # 2 u-tiles
NQ = QS // 512             # 4 matmul n-chunks per core

F8 = ml_dtypes.float8_e4m3
BF = ml_dtypes.bfloat16

_PROG_CACHE = {}


def _build_program(M: float):
    import concourse.bacc as bacc
    import concourse.tile as tile
    from concourse import mybir
    from concourse.masks import make_identity

    f32 = mybir.dt.float32
    bf16 = mybir.dt.bfloat16
    fp8 = mybir.dt.float8e4
    AF = mybir.ActivationFunctionType
    OP = mybir.AluOpType
    DR = mybir.MatmulPerfMode.DoubleRow

    nc = bacc.Bacc("TRN2", target_bir_lowering=False, debug=False,
                   num_devices=N_CORES)

    xT8_d = nc.dram_tensor("xT8", [128, KP * KI * B], fp8, kind="ExternalInput").ap()
    x8b_d = nc.dram_tensor("x8b", [128, MT * D], fp8, kind="ExternalInput").ap()
    mk8_d = nc.dram_tensor("mk8", [128, MT * U], fp8, kind="ExternalInput").ap()
    eT8_d = nc.dram_tensor("eT8", [128, KP * KI * QS], fp8, kind="ExternalInput").ap()
    rcnt_d = nc.dram_tensor("rcntf", [128, UT], f32, kind="ExternalInput").ap()
    gsx_d = nc.dram_tensor("gsxe", [128, 1], f32, kind="ExternalInput").ap()
    ukp_d = nc.dram_tensor("ukeepf", [128, UT], f32, kind="ExternalInput").ap()
    widx_d = nc.dram_tensor("widxf", [128, MT], f32, kind="ExternalInput").ap()
    iota_d = nc.dram_tensor("iotau", [128, U], f32, kind="ExternalInput").ap()
    pidx_d = nc.dram_tensor("pidxf", [128, 1], f32, kind="ExternalInput").ap()
    osum_d = nc.dram_tensor("osum", [128, MT], f32, kind="ExternalOutput").ap()
    tco_d = nc.dram_tensor("tco", [128, MT], f32, kind="ExternalOutput").ap()

    with tile.TileContext(nc) as tc:
        with (
            tc.tile_pool(name="singles", bufs=1) as singles,
            tc.tile_pool(name="work", bufs=3) as work,
            tc.tile_pool(name="small", bufs=4) as small,
        ):
            ident = singles.tile([128, 128], bf16)
            make_identity(nc, ident)

            # big resident operands first: the DMA pipe drains in issue order
            xT8 = singles.tile([128, KP, KI, B], fp8)
            nc.sync.dma_start(out=xT8, in_=xT8_d)
            x8b = singles.tile([128, MT, D], fp8)
            for h in range(4):
                s = MT // 4 * h
                nc.sync.dma_start(out=x8b[:, s:s + MT // 4, :],
                                  in_=x8b_d[:, s * D:(s + MT // 4) * D])
            mk8 = singles.tile([128, MT, U], fp8)
            nc.sync.dma_start(out=mk8, in_=mk8_d)
            eT8 = singles.tile([128, KP, KI, QS], fp8)
            nc.sync.dma_start(out=eT8, in_=eT8_d)

            rcnt = singles.tile([128, UT], f32)
            nc.sync.dma_start(out=rcnt, in_=rcnt_d)
            gsx = singles.tile([128, 1], f32)
            nc.sync.dma_start(out=gsx, in_=gsx_d)
            ukp = singles.tile([128, UT], f32)
            nc.sync.dma_start(out=ukp, in_=ukp_d)
            widx = singles.tile([128, MT], f32)
            nc.sync.dma_start(out=widx, in_=widx_d)
            iotab = singles.tile([128, U], f32)
            nc.sync.dma_start(out=iotab, in_=iota_d)
            pidx = singles.tile([128, 1], f32)
            nc.sync.dma_start(out=pidx, in_=pidx_d)

            biasM = singles.tile([128, 1], f32)
            nc.vector.memset(biasM, -M)

            uembT8 = singles.tile([128, KP, KI, U], fp8)   # gated fp8 means^T
            nsq = singles.tile([128, MT], f32)             # |x_b|^2
            rin30 = singles.tile([128, MT], f32)           # 30/(SXE*|x_b|)
            macc = singles.tile([128, UT], f32)            # |mean_u|^2
            osum = singles.tile([128, MT], f32)            # sum-exp collector
            tco = singles.tile([128, MT], f32)             # target-cos collector

            # ---------- phase A: row norms + masked means ----------
            with (
                tc.tile_pool(name="psum_u", bufs=1, space="PSUM") as psum_u,
                tc.tile_pool(name="psum_n", bufs=4, space="PSUM") as psum_n,
            ):
                # norms: diag(x x^T) per b-tile via fp8-DR, diag gather on DVE
                for m in range(MT):
                    psn = psum_n.tile([128, 512], f32, tag="psn")
                    dv = psn[:, 0:128]
                    xs = xT8[:, :, :, m * 128:(m + 1) * 128]
                    for kp in range(KP):
                        nc.tensor.matmul(dv, xs[:, kp, :, :], xs[:, kp, :, :],
                                         start=(kp == 0), stop=(kp == KP - 1),
                                         perf_mode=DR)
                    scr = work.tile([128, 128], f32, tag="dscr")
                    nc.vector.scalar_tensor_tensor(
                        out=scr, in0=iotab[:, 0:128], scalar=pidx[:, 0:1],
                        in1=dv, op0=OP.is_equal, op1=OP.mult,
                        accum_out=nsq[:, m:m + 1])

                # means: fp8-DR over b-pairs (host-built one-hot masks)
                ps_u = [psum_u.tile([128, D], f32, tag=f"uniq{mu}",
                                    name=f"ps_u{mu}") for mu in range(UT)]
                for t in range(MT // 2):
                    for mu in range(UT):
                        nc.tensor.matmul(
                            ps_u[mu],
                            mk8[:, 2 * t:2 * t + 2, mu * 128:(mu + 1) * 128],
                            x8b[:, 2 * t:2 * t + 2, :],
                            start=(t == 0), stop=(t == MT // 2 - 1),
                            perf_mode=DR)

                # norm finalize (batched)
                nrm = small.tile([128, MT], f32, tag="nrm")
                nc.scalar.activation(out=nrm, in_=nsq, func=AF.Sqrt)
                nc.vector.tensor_scalar_max(out=nrm, in0=nrm, scalar1=1e-12)
                rinv = small.tile([128, MT], f32, tag="rinv")
                nc.vector.reciprocal(rinv, nrm)
                nc.vector.tensor_scalar_mul(out=rin30, in0=rinv,
                                            scalar1=OIM_SCALAR / SXE)

                # means: psum -> bf16 (ACT copy w/ per-partition 1/cnt scale)
                mean_bf = [singles.tile([128, D], bf16, name=f"mean_bf{mu}")
                           for mu in range(UT)]
                for mu in range(UT):
                    nc.scalar.activation(out=mean_bf[mu], in_=ps_u[mu],
                                         func=AF.Copy,
                                         scale=rcnt[:, mu:mu + 1])
                    sq2 = work.tile([128, D], bf16, tag="sq")
                    nc.scalar.activation(out=sq2, in_=mean_bf[mu],
                                         func=AF.Square,
                                         accum_out=macc[:, mu:mu + 1])

            # ---------- mean normalize + transpose to fp8 d-major ----------
            with tc.tile_pool(name="psum_t", bufs=2, space="PSUM") as psum_t:
                mnr = small.tile([128, UT], f32, tag="mnr")
                nc.scalar.activation(out=mnr, in_=macc, func=AF.Sqrt)
                nc.vector.tensor_scalar_max(out=mnr, in0=mnr, scalar1=1e-12)
                mrc = small.tile([128, UT], f32, tag="mrc")
                nc.vector.reciprocal(mrc, mnr)
                # rmg = SXE*gate*ukeep/|mean| (per-partition u scale)
                rmg = small.tile([128, UT], f32, tag="rmg")
                nc.vector.tensor_scalar(out=rmg, in0=mrc,
                                        scalar1=gsx[:, 0:1], scalar2=None,
                                        op0=OP.mult)
                nc.vector.tensor_tensor(out=rmg, in0=rmg, in1=ukp,
                                        op=OP.mult)
                for mu in range(UT):
                    mng = work.tile([128, D], bf16, tag="mng")
                    nc.vector.tensor_scalar_mul(out=mng, in0=mean_bf[mu],
                                                scalar1=rmg[:, mu:mu + 1])
                    pst = psum_t.tile([128, D], bf16, tag="pst")
                    for kd in range(D // 128):
                        nc.tensor.transpose(pst[:, kd * 128:(kd + 1) * 128],
                                            mng[:, kd * 128:(kd + 1) * 128],
                                            ident)
                    nc.vector.tensor_scalar_mul(
                        out=uembT8[:, :, :, mu * 128:(mu + 1) * 128],
                        in0=pst, scalar1=1.0)
                # merge gated means into the first W0 columns of eT8
                ev = eT8[:, :, :, 0:U]
                nc.vector.scalar_tensor_tensor(
                    out=ev, in0=uembT8, scalar=1.0, in1=ev,
                    op0=OP.mult, op1=OP.add)

            # ---------- phase C: logits + exp + sums + target gather -------
            with tc.tile_pool(name="psum_l", bufs=2, space="PSUM") as psum_l:
                for m in range(MT):
                    pl = psum_l.tile([128, NQ * 512], f32, tag="pl")
                    for n in range(NQ):
                        for kp in range(KP):
                            nc.tensor.matmul(
                                pl[:, n * 512:(n + 1) * 512],
                                xT8[:, kp, :, m * 128:(m + 1) * 128],
                                eT8[:, kp, :, n * 512:(n + 1) * 512],
                                start=(kp == 0), stop=(kp == KP - 1),
                                perf_mode=DR)
                    # target logit: window cols live at 0:U (core 0 layout)
                    scr = work.tile([128, U], f32, tag="scr")
                    nc.vector.scalar_tensor_tensor(
                        out=scr, in0=iotab, scalar=widx[:, m:m + 1],
                        in1=pl[:, 0:U], op0=OP.is_equal, op1=OP.mult,
                        accum_out=tco[:, m:m + 1])
                    ex = work.tile([128, NQ * 512], bf16, tag="ex", bufs=2)
                    nc.scalar.activation(out=ex, in_=pl, func=AF.Exp,
                                         bias=biasM, scale=rin30[:, m:m + 1])
                    nc.vector.tensor_scalar(out=ex, in0=ex, scalar1=1.0,
                                            scalar2=0.0, op0=OP.mult,
                                            op1=OP.add,
                                            accum_out=osum[:, m:m + 1])
                nc.vector.tensor_tensor(out=tco, in0=tco, in1=rin30,
                                        op=OP.mult)

            nc.sync.dma_start(out=osum_d, in_=osum)
            nc.sync.dma_start(out=tco_d, in_=tco)

    nc.compile()
    return nc


def _host_bookkeeping(labels, label_cq, header_cq):
    """Mirror the reference's integer-only queue-update semantics."""
    labels = np.asarray(labels).astype(np.int64)
    lab = np.asarray(label_cq).astype(np.int64).copy()
    h0 = int(np.asarray(header_cq))

    uq = np.unique(labels)
    if uq.size < U:
        uniq = np.concatenate([uq, np.full(U - uq.size, uq.min(), np.int64)])
    else:
        uniq = uq[:U]
    cnts = np.array([(labels == v).sum() for v in uniq], np.int64)

    emb_src = np.full(Q, -1, np.int64)   # >=0: row u of uniq means; -1: original
    h = h0 % Q
    for u in range(U):
        y = uniq[u]
        m = lab == y
        i = int(np.argmax(m)) if m.any() else 0
        inval = bool(m.any()) and (i != h)
        emb_src[h] = u
        lab[h] = y
        if inval:
            lab[i] = IGNORE
        h = (h + 1) % Q

    good = lab != IGNORE
    goodidx = np.flatnonzero(good)
    gl = lab[goodidx]
    vals, first = np.unique(gl, return_index=True)
    pos = np.searchsorted(vals, labels)
    assert np.all(vals[np.clip(pos, 0, vals.size - 1)] == labels), \
        "batch label missing from queue"
    xe = goodidx[first[pos]]
    return uniq, cnts, emb_src, good, xe


def _pmajor(v, cols):
    return np.ascontiguousarray(np.asarray(v, np.float32).reshape(cols, 128).T)


def _prepare(inputs, labels, emb_cq, label_cq, header_cq):
    inputs = np.asarray(inputs, np.float32)
    emb_cq = np.asarray(emb_cq, np.float32)
    labels = np.asarray(labels)

    uniq, cnts, emb_src, good, xe = _host_bookkeeping(labels, label_cq, header_cq)

    max_nrm = float(np.sqrt((emb_cq.astype(np.float64) ** 2).sum(axis=1).max()))
    M = OIM_SCALAR * max(1.0, max_nrm) * 1.0000001

    window = emb_src >= 0
    u_slot = np.full(U, -1, np.int64)
    wi = np.flatnonzero(window)
    u_slot[emb_src[wi]] = wi
    u_kept = (u_slot >= 0) & good[np.clip(u_slot, 0, Q - 1)]

    w_idx = emb_src[xe].astype(np.float64)        # -1 for non-window targets
    w_idx[w_idx >= 0] = np.where(
        u_kept[w_idx[w_idx >= 0].astype(np.int64)],
        w_idx[w_idx >= 0], -1.0)
    extra = np.flatnonzero(w_idx < 0)             # handled on host (rare/none)

    # ---- device input layouts ----
    x8 = inputs.astype(F8)
    x8b = np.ascontiguousarray(
        x8.reshape(MT, 128, D).transpose(1, 0, 2).reshape(128, MT * D))
    # xT8[p, kp, i, b] = fp8(x[b, 256*kp + 128*i + p])
    xT8 = np.ascontiguousarray(
        x8.T.reshape(KP, KI, 128, B).transpose(2, 0, 1, 3).reshape(128, -1))
    # mk8[p, m, u] = (labels[128m+p] == uniq[u])
    mk8 = (labels.reshape(MT, 128).T[:, :, None] == uniq[None, None, :])
    mk8 = np.ascontiguousarray(mk8.astype(F8).reshape(128, MT * U))

    keep_orig = good & ~window
    embq = (SXE * emb_cq).astype(F8)
    embq[~keep_orig] = 0                          # bad or window -> zero cols
    orig_idx = np.flatnonzero(~window)            # Q-U slots, canonical order
    n_orig0 = QS - W0                             # originals on core 0

    nzero = int((~keep_orig[orig_idx]).sum()) + int((~u_kept).sum())

    base = {
        "xT8": xT8,
        "x8b": x8b,
        "mk8": mk8,
        "rcntf": _pmajor(1.0 / cnts.astype(np.float64), UT),
        "ukeepf": _pmajor(u_kept.astype(np.float64), UT),
        "widxf": _pmajor(w_idx, MT),
        "iotau": np.ascontiguousarray(
            np.broadcast_to(np.arange(U, dtype=np.float32), (128, U))),
        "pidxf": np.arange(128, dtype=np.float32).reshape(128, 1),
    }

    def to_dmajor(cols):
        # cols: [QS, D] fp8 -> [128, KP*KI*QS] with (p,kp,i,j) layout
        t = np.ascontiguousarray(cols).T          # [D, QS]
        return np.ascontiguousarray(
            t.reshape(KP, KI, 128, QS).transpose(2, 0, 1, 3).reshape(128, -1))

    in_maps = []
    for c in range(N_CORES):
        cols = np.zeros((QS, D), F8)
        if c == 0:
            cols[W0:] = embq[orig_idx[:n_orig0]]
        else:
            sl = orig_idx[n_orig0 + (c - 1) * QS: n_orig0 + c * QS]
            cols[:] = embq[sl]
        gate = np.full((128, 1), SXE if c == 0 else 0.0, np.float32)
        in_maps.append({**base, "eT8": to_dmajor(cols), "gsxe": gate})
    return M, in_maps, extra, xe, nzero


def _combine(res_list, M, extra, xe, nzero, inputs, emb_cq):
    S = np.zeros(B, np.float64)
    for r in res_list:
        S += r["osum"].astype(np.float64).T.reshape(B)
    S -= nzero * np.exp(-np.float64(M))
    t30 = res_list[0]["tco"].astype(np.float64).T.reshape(B)

    if extra.size:  # targets pointing at original (non-window) queue rows
        xb = np.asarray(inputs, np.float64)[extra]
        xb /= np.maximum(np.linalg.norm(xb, axis=1, keepdims=True), 1e-12)
        eb = np.asarray(emb_cq, np.float64)[xe[extra]]
        t30[extra] = OIM_SCALAR * (xb * eb).sum(axis=1)

    loss = np.mean(M + np.log(S) - t30)
    return np.array(loss, dtype=np.float32)


def kernel(inputs, labels, emb_cq, label_cq, age_cq, header_cq):
    from concourse.bass_utils import run_bass_kernel_spmd

    M, in_maps, extra, xe, nzero = _prepare(
        inputs, labels, emb_cq, label_cq, header_cq)

    key = round(M, 9)
    if key not in _PROG_CACHE:
        _PROG_CACHE[key] = _build_program(M)
    nc = _PROG_CACHE[key]

    res = run_bass_kernel_spmd(nc, in_maps, core_ids=list(range(N_CORES)))
    return _combine(res.results, M, extra, xe, nzero, inputs, emb_cq)


# revision 15
# speedup vs baseline: 2.6583x; 1.0973x over previous
"""OIM loss with circular queue — Trainium2 Bass kernel (8 NeuronCores).

Strategy (v3, fp8 DoubleRow end-to-end)
---------------------------------------
loss = mean_b [ M + log S_b - 30*cos(x_b, e_{xe_b}) ],
S_b = sum_{q good} exp(30*cos(x_b, e_q) - M), with e the post-update queue.

Device-side compute per core (tensor-parallel over Q):
  - per-pid masked means: fp8 DoubleRow matmul (host-built one-hot masks)
  - row norms of x: fp8 DR matmul x@x^T per b-tile, diagonal extracted on DVE
  - normalized means -> gated fp8 d-major tiles (PE transpose + cast)
  - big logits matmul in fp8e4 DoubleRow (K=256/pass) into [128,2048] PSUM,
    exp on ACT (per-row scale=30*rin/SX, bias=-M), row-sum accumulated by a
    DVE tensor_scalar pass, target logits gathered from PSUM cols 0:256.

Layout trick: the 256 queue slots rewritten by the circular-queue update
("window") are core 0's first 256 columns; the other 16128 original slots
fill the rest (2048/core on cores 1-7).  Bad slots (label IGNORE) are zero
columns -> each contributes exactly exp(-M), subtracted on the host.  The
target slot of every batch row is a window slot, so its logit is read from
the first 256 PSUM columns on core 0 (cores 1-7 produce garbage there and
the host ignores it).  All integer bookkeeping, input layout (fp8
quantization / transposes) and the final log/mean run on the host; all
O(B*D*Q) FLOPs run on device.
"""

import os
import sys

import numpy as np

for _p in ("/opt/trn_rl_repo", "/root/.axon_site/_ro/trn_rl_repo"):
    if os.path.isdir(_p) and _p not in sys.path:
        sys.path.insert(0, _p)

import ml_dtypes

B, D, Q, U = 4096, 512, 16384, 256
N_CORES = 8
QS = Q // N_CORES          # queue columns per core
W0 = U                     # window block size on core 0's layout
OIM_SCALAR = 30.0
IGNORE = -1
SXE = 16.0                 # fp8 scale for emb/mean operands
MT = B // 128              # 32 b-tiles
KP = 2                     # k-passes of 256 (DoubleRow)
KI = 2                     # interleave factor inside a pass
UT = U // 128              # 2 u-tiles
NQ = QS // 512             # 4 matmul n-chunks per core

F8 = ml_dtypes.float8_e4m3
BF = ml_dtypes.bfloat16

_PROG_CACHE = {}


def _build_program(M: float):
    import concourse.bacc as bacc
    import concourse.tile as tile
    from concourse import mybir
    from concourse.masks import make_identity

    f32 = mybir.dt.float32
    bf16 = mybir.dt.bfloat16
    fp8 = mybir.dt.float8e4
    AF = mybir.ActivationFunctionType
    OP = mybir.AluOpType
    DR = mybir.MatmulPerfMode.DoubleRow

    nc = bacc.Bacc("TRN2", target_bir_lowering=False, debug=False,
                   num_devices=N_CORES)

    xT8_d = nc.dram_tensor("xT8", [128, KP, KI, B], fp8, kind="ExternalInput").ap()
    x8b_d = nc.dram_tensor("x8b", [128, MT * D], fp8, kind="ExternalInput").ap()
    mk8_d = nc.dram_tensor("mk8", [128, MT * U], fp8, kind="ExternalInput").ap()
    eT8_d = nc.dram_tensor("eT8", [128, KP * KI * QS], fp8, kind="ExternalInput").ap()
    rcnt_d = nc.dram_tensor("rcntf", [128, UT], f32, kind="ExternalInput").ap()
    gukp_d = nc.dram_tensor("gukp", [128, UT], f32, kind="ExternalInput").ap()
    widx_d = nc.dram_tensor("widxf", [128, MT], f32, kind="ExternalInput").ap()
    iota_d = nc.dram_tensor("iotau", [128, U], f32, kind="ExternalInput").ap()
    pidx_d = nc.dram_tensor("pidxf", [128, 1], f32, kind="ExternalInput").ap()
    osum_d = nc.dram_tensor("osum", [128, MT], f32, kind="ExternalOutput").ap()
    tco_d = nc.dram_tensor("tco", [128, MT], f32, kind="ExternalOutput").ap()

    with tile.TileContext(nc) as tc:
        with (
            tc.tile_pool(name="singles", bufs=1) as singles,
            tc.tile_pool(name="work", bufs=3) as work,
            tc.tile_pool(name="small", bufs=4) as small,
        ):
            ident = singles.tile([128, 128], bf16)
            make_identity(nc, ident)

            # small inputs first (the DMA pipe drains in issue order), then
            # the big operands interleaved to unblock compute ASAP
            rcnt = singles.tile([128, UT], f32)
            nc.sync.dma_start(out=rcnt, in_=rcnt_d)
            gukp = singles.tile([128, UT], f32)
            nc.sync.dma_start(out=gukp, in_=gukp_d)
            widx = singles.tile([128, MT], f32)
            nc.sync.dma_start(out=widx, in_=widx_d)
            iotab = singles.tile([128, U], f32)
            nc.sync.dma_start(out=iotab, in_=iota_d)
            pidx = singles.tile([128, 1], f32)
            nc.sync.dma_start(out=pidx, in_=pidx_d)

            BC = B // 4   # b-range per xT8/x8b DMA chunk
            xT8 = singles.tile([128, KP, KI, B], fp8)
            x8b = singles.tile([128, MT, D], fp8)
            mk8 = singles.tile([128, MT, U], fp8)
            eT8 = singles.tile([128, KP, KI, QS], fp8)

            def xT8_chunk(h):
                nc.sync.dma_start(out=xT8[:, :, :, h * BC:(h + 1) * BC],
                                  in_=xT8_d[:, :, :, h * BC:(h + 1) * BC])

            def x8b_chunk(h):
                s = MT // 4 * h
                nc.sync.dma_start(out=x8b[:, s:s + MT // 4, :],
                                  in_=x8b_d[:, s * D:(s + MT // 4) * D])

            xT8_chunk(0)
            nc.sync.dma_start(out=mk8, in_=mk8_d)
            x8b_chunk(0)
            for h in range(1, 4):
                xT8_chunk(h)
                x8b_chunk(h)
            nc.sync.dma_start(out=eT8, in_=eT8_d)

            biasM = singles.tile([128, 1], f32)
            nc.vector.memset(biasM, -M)
            epsb = singles.tile([128, 1], f32)
            nc.vector.memset(epsb, 1e-24)

            uembT8 = singles.tile([128, KP, KI, U], fp8)   # gated fp8 means^T
            nsq = singles.tile([128, MT], f32)             # |x_b|^2
            rin30 = singles.tile([128, MT], f32)           # 30/(SXE*|x_b|)
            macc = singles.tile([128, UT], f32)            # |mean_u|^2
            osum = singles.tile([128, MT], f32)            # sum-exp collector
            tco = singles.tile([128, MT], f32)             # target-cos collector

            # ---------- phase A: masked means + row norms ----------
            with (
                tc.tile_pool(name="psum_u", bufs=1, space="PSUM") as psum_u,
                tc.tile_pool(name="psum_n", bufs=4, space="PSUM") as psum_n,
            ):
                # means: fp8-DR over b-pairs (host-built one-hot masks)
                ps_u = [psum_u.tile([128, D], f32, tag=f"uniq{mu}",
                                    name=f"ps_u{mu}") for mu in range(UT)]
                for t in range(MT // 2):
                    for mu in range(UT):
                        nc.tensor.matmul(
                            ps_u[mu],
                            mk8[:, 2 * t:2 * t + 2, mu * 128:(mu + 1) * 128],
                            x8b[:, 2 * t:2 * t + 2, :],
                            start=(t == 0), stop=(t == MT // 2 - 1),
                            perf_mode=DR)

                # norms: diag(x x^T) per b-tile via fp8-DR, diag gathered on
                # DVE/Pool (alternating)
                for m in range(MT):
                    psn = psum_n.tile([128, 512], f32, tag="psn")
                    dv = psn[:, 0:128]
                    xs = xT8[:, :, :, m * 128:(m + 1) * 128]
                    for kp in range(KP):
                        nc.tensor.matmul(dv, xs[:, kp, :, :], xs[:, kp, :, :],
                                         start=(kp == 0), stop=(kp == KP - 1),
                                         perf_mode=DR)
                    scr = work.tile([128, 128], f32, tag="dscr")
                    nc.vector.scalar_tensor_tensor(
                        out=scr, in0=iotab[:, 0:128], scalar=pidx[:, 0:1],
                        in1=dv, op0=OP.is_equal, op1=OP.mult,
                        accum_out=nsq[:, m:m + 1])

                # norm finalize: rin30 = 30/(SXE*|x|) via scaled sqrt+recip
                nrm = small.tile([128, MT], f32, tag="nrm")
                nc.scalar.activation(out=nrm, in_=nsq, func=AF.Sqrt,
                                     bias=epsb, scale=(SXE / OIM_SCALAR) ** 2)
                nc.vector.reciprocal(rin30, nrm)

                # means: psum -> bf16 + squared norm (both read ps_u on ACT)
                mean_bf = [singles.tile([128, D], bf16, name=f"mean_bf{mu}")
                           for mu in range(UT)]
                for mu in range(UT):
                    nc.scalar.activation(out=mean_bf[mu], in_=ps_u[mu],
                                         func=AF.Copy,
                                         scale=rcnt[:, mu:mu + 1])
                    sq2 = work.tile([128, D], bf16, tag="sq")
                    nc.scalar.activation(out=sq2, in_=ps_u[mu],
                                         func=AF.Square,
                                         scale=rcnt[:, mu:mu + 1],
                                         accum_out=macc[:, mu:mu + 1])

            # ---------- mean normalize + transpose to fp8 d-major ----------
            with tc.tile_pool(name="psum_t", bufs=2, space="PSUM") as psum_t:
                mnr = small.tile([128, UT], f32, tag="mnr")
                nc.scalar.activation(out=mnr, in_=macc, func=AF.Sqrt,
                                     bias=epsb)
                mrc = small.tile([128, UT], f32, tag="mrc")
                nc.vector.reciprocal(mrc, mnr)
                # rmg = SXE*gate*ukeep/|mean| (per-partition u scale)
                rmg = small.tile([128, UT], f32, tag="rmg")
                nc.vector.tensor_tensor(out=rmg, in0=mrc, in1=gukp,
                                        op=OP.mult)
                for mu in range(UT):
                    mng = work.tile([128, D], bf16, tag="mng")
                    nc.vector.tensor_scalar_mul(out=mng, in0=mean_bf[mu],
                                                scalar1=rmg[:, mu:mu + 1])
                    pst = psum_t.tile([128, D], bf16, tag="pst")
                    for kd in range(D // 128):
                        nc.tensor.transpose(pst[:, kd * 128:(kd + 1) * 128],
                                            mng[:, kd * 128:(kd + 1) * 128],
                                            ident)
                    nc.vector.tensor_scalar_mul(
                        out=uembT8[:, :, :, mu * 128:(mu + 1) * 128],
                        in0=pst, scalar1=1.0)
                # merge gated means into the first W0 columns of eT8
                ev = eT8[:, :, :, 0:U]
                nc.vector.scalar_tensor_tensor(
                    out=ev, in0=uembT8, scalar=1.0, in1=ev,
                    op0=OP.mult, op1=OP.add)

            # ---------- phase C: logits + exp + sums + target gather -------
            with tc.tile_pool(name="psum_l", bufs=2, space="PSUM") as psum_l:
                for m in range(MT):
                    pl = psum_l.tile([128, NQ * 512], f32, tag="pl")
                    for n in range(NQ):
                        for kp in range(KP):
                            nc.tensor.matmul(
                                pl[:, n * 512:(n + 1) * 512],
                                xT8[:, kp, :, m * 128:(m + 1) * 128],
                                eT8[:, kp, :, n * 512:(n + 1) * 512],
                                start=(kp == 0), stop=(kp == KP - 1),
                                perf_mode=DR)
                    # target logit: window cols live at 0:U (core 0 layout)
                    scr = work.tile([128, U], f32, tag="scr")
                    nc.vector.scalar_tensor_tensor(
                        out=scr, in0=iotab, scalar=widx[:, m:m + 1],
                        in1=pl[:, 0:U], op0=OP.is_equal, op1=OP.mult,
                        accum_out=tco[:, m:m + 1])
                    ex = work.tile([128, NQ * 512], bf16, tag="ex", bufs=2)
                    nc.scalar.activation(out=ex, in_=pl, func=AF.Exp,
                                         bias=biasM, scale=rin30[:, m:m + 1])
                    nc.vector.tensor_scalar(out=ex, in0=ex, scalar1=1.0,
                                            scalar2=0.0, op0=OP.mult,
                                            op1=OP.add,
                                            accum_out=osum[:, m:m + 1])
                nc.vector.tensor_tensor(out=tco, in0=tco, in1=rin30,
                                        op=OP.mult)

            nc.sync.dma_start(out=osum_d, in_=osum)
            nc.sync.dma_start(out=tco_d, in_=tco)

    nc.compile()
    return nc


def _host_bookkeeping(labels, label_cq, header_cq):
    """Mirror the reference's integer-only queue-update semantics."""
    labels = np.asarray(labels).astype(np.int64)
    lab = np.asarray(label_cq).astype(np.int64).copy()
    h0 = int(np.asarray(header_cq))

    uq = np.unique(labels)
    if uq.size < U:
        uniq = np.concatenate([uq, np.full(U - uq.size, uq.min(), np.int64)])
    else:
        uniq = uq[:U]
    cnts = np.array([(labels == v).sum() for v in uniq], np.int64)

    emb_src = np.full(Q, -1, np.int64)   # >=0: row u of uniq means; -1: original
    h = h0 % Q
    for u in range(U):
        y = uniq[u]
        m = lab == y
        i = int(np.argmax(m)) if m.any() else 0
        inval = bool(m.any()) and (i != h)
        emb_src[h] = u
        lab[h] = y
        if inval:
            lab[i] = IGNORE
        h = (h + 1) % Q

    good = lab != IGNORE
    goodidx = np.flatnonzero(good)
    gl = lab[goodidx]
    vals, first = np.unique(gl, return_index=True)
    pos = np.searchsorted(vals, labels)
    assert np.all(vals[np.clip(pos, 0, vals.size - 1)] == labels), \
        "batch label missing from queue"
    xe = goodidx[first[pos]]
    return uniq, cnts, emb_src, good, xe


def _pmajor(v, cols):
    return np.ascontiguousarray(np.asarray(v, np.float32).reshape(cols, 128).T)


def _prepare(inputs, labels, emb_cq, label_cq, header_cq):
    inputs = np.asarray(inputs, np.float32)
    emb_cq = np.asarray(emb_cq, np.float32)
    labels = np.asarray(labels)

    uniq, cnts, emb_src, good, xe = _host_bookkeeping(labels, label_cq, header_cq)

    max_nrm = float(np.sqrt((emb_cq.astype(np.float64) ** 2).sum(axis=1).max()))
    M = OIM_SCALAR * max(1.0, max_nrm) * 1.0000001

    window = emb_src >= 0
    u_slot = np.full(U, -1, np.int64)
    wi = np.flatnonzero(window)
    u_slot[emb_src[wi]] = wi
    u_kept = (u_slot >= 0) & good[np.clip(u_slot, 0, Q - 1)]

    w_idx = emb_src[xe].astype(np.float64)        # -1 for non-window targets
    w_idx[w_idx >= 0] = np.where(
        u_kept[w_idx[w_idx >= 0].astype(np.int64)],
        w_idx[w_idx >= 0], -1.0)
    extra = np.flatnonzero(w_idx < 0)             # handled on host (rare/none)

    # ---- device input layouts ----
    x8 = inputs.astype(F8)
    x8b = np.ascontiguousarray(
        x8.reshape(MT, 128, D).transpose(1, 0, 2).reshape(128, MT * D))
    # xT8[p, kp, i, b] = fp8(x[b, 256*kp + 128*i + p])
    xT8 = np.ascontiguousarray(
        x8.T.reshape(KP, KI, 128, B).transpose(2, 0, 1, 3).reshape(128, -1))
    # mk8[p, m, u] = (labels[128m+p] == uniq[u])
    mk8 = (labels.reshape(MT, 128).T[:, :, None] == uniq[None, None, :])
    mk8 = np.ascontiguousarray(mk8.astype(F8).reshape(128, MT * U))

    keep_orig = good & ~window
    embq = (SXE * emb_cq).astype(F8)
    embq[~keep_orig] = 0                          # bad or window -> zero cols
    orig_idx = np.flatnonzero(~window)            # Q-U slots, canonical order
    n_orig0 = QS - W0                             # originals on core 0

    nzero = int((~keep_orig[orig_idx]).sum()) + int((~u_kept).sum())

    base = {
        "xT8": xT8.reshape(128, KP, KI, B),
        "x8b": x8b,
        "mk8": mk8,
        "rcntf": _pmajor(1.0 / cnts.astype(np.float64), UT),
        "widxf": _pmajor(w_idx, MT),
        "iotau": np.ascontiguousarray(
            np.broadcast_to(np.arange(U, dtype=np.float32), (128, U))),
        "pidxf": np.arange(128, dtype=np.float32).reshape(128, 1),
    }
    gukp_full = _pmajor(SXE * u_kept.astype(np.float64), UT)
    gukp_zero = np.zeros((128, UT), np.float32)

    def to_dmajor(cols):
        # cols: [QS, D] fp8 -> [128, KP*KI*QS] with (p,kp,i,j) layout
        t = np.ascontiguousarray(cols).T          # [D, QS]
        return np.ascontiguousarray(
            t.reshape(KP, KI, 128, QS).transpose(2, 0, 1, 3).reshape(128, -1))

    in_maps = []
    for c in range(N_CORES):
        cols = np.zeros((QS, D), F8)
        if c == 0:
            cols[W0:] = embq[orig_idx[:n_orig0]]
        else:
            sl = orig_idx[n_orig0 + (c - 1) * QS: n_orig0 + c * QS]
            cols[:] = embq[sl]
        in_maps.append({**base, "eT8": to_dmajor(cols),
                        "gukp": gukp_full if c == 0 else gukp_zero})
    return M, in_maps, extra, xe, nzero


def _combine(res_list, M, extra, xe, nzero, inputs, emb_cq):
    S = np.zeros(B, np.float64)
    for r in res_list:
        S += r["osum"].astype(np.float64).T.reshape(B)
    S -= nzero * np.exp(-np.float64(M))
    t30 = res_list[0]["tco"].astype(np.float64).T.reshape(B)

    if extra.size:  # targets pointing at original (non-window) queue rows
        xb = np.asarray(inputs, np.float64)[extra]
        xb /= np.maximum(np.linalg.norm(xb, axis=1, keepdims=True), 1e-12)
        eb = np.asarray(emb_cq, np.float64)[xe[extra]]
        t30[extra] = OIM_SCALAR * (xb * eb).sum(axis=1)

    loss = np.mean(M + np.log(S) - t30)
    return np.array(loss, dtype=np.float32)


def kernel(inputs, labels, emb_cq, label_cq, age_cq, header_cq):
    from concourse.bass_utils import run_bass_kernel_spmd

    M, in_maps, extra, xe, nzero = _prepare(
        inputs, labels, emb_cq, label_cq, header_cq)

    key = round(M, 9)
    if key not in _PROG_CACHE:
        _PROG_CACHE[key] = _build_program(M)
    nc = _PROG_CACHE[key]

    res = run_bass_kernel_spmd(nc, in_maps, core_ids=list(range(N_CORES)))
    return _combine(res.results, M, extra, xe, nzero, inputs, emb_cq)
